# revision 67
# baseline (speedup 1.0000x reference)
"""Causal self-attention (B=4, T=2048, C=768, H=12) on 8 trn2 NeuronCores.

Sharding: 8 cores = 4 batches x 2 head-groups (6 heads each).
Each core: QKV projection for its 6 heads, causal attention, partial output
projection (row-parallel). Host sums the two partials per batch + b_proj.

Device schedule:
  - Q/K projection runs in fp8 e4m3 with MatmulPerfMode.DoubleRow
    (256-deep contraction per matmul, 2x bf16 throughput); operands are
    host-packed so every DoubleRow lhsT/rhs is a contiguous SBUF slice.
    V'/scores/AV/proj stay bf16 (fp8 there fails the accuracy budget).
  - Scores for a head PAIR share one [128,1024] PSUM tile (2 banks); one
    merged exp per off-diagonal block covers both heads.
  - PSUM: 2x score pair tiles (4 banks) + 2x single-bank filler tiles
    (QKV/V'/proj groups) + 2 AV accumulator banks.
  - AV matmuls run two iterations behind their scores so the exp/mask
    latency never stalls the PE; all non-attention matmul work is queued
    as closures and drip-fed between attention iterations (a small
    reserve is popped just before each chunk's normalize chain so the PE
    stays busy through it; the final chain runs on ACT + a PE ones-matmul
    broadcast instead of the slow gpsimd path).
  - softmax denominators come free via a ones-column appended to V'
    (65-row AV output); normalize = copy to SBUF (custom-DVE reciprocal
    needs base partition 0 + SBUF), reciprocal, partition-broadcast, mul.
  - Output stored bf16 (host accumulates the two partials in fp32).
"""

import math
import os
import sys
import types
from collections import deque

sys.path.insert(0, "/opt/trn_rl_repo")

import ml_dtypes
import numpy as np

import concourse.bass as bass
import concourse.tile as tile
from concourse import bacc, mybir
from concourse.bass_utils import run_bass_kernel_spmd

B, T, C, H, D = 4, 2048, 768, 12, 64
N_CORES = 8
HPC = H // 2          # heads per core = 6
FQK = 2 * HPC * D     # 768 qk features per core
FV = HPC * D          # 384 v features per core
E = D + 1             # 65: head dim + ones column
TT = T // 128         # 16 token tiles
CCH = C // 128        # 6 contraction chunks
QC = T // 512         # 4 query chunks of 512
F32 = mybir.dt.float32
BF16 = mybir.dt.bfloat16
FP8 = mybir.dt.float8e4
NPBF = ml_dtypes.bfloat16
NPF8 = ml_dtypes.float8_e4m3


def _install_ntff_hook():
    """The image's antenv lacks axon_hooks; inject it so trace=True works."""
    if "antenv.axon_hooks" in sys.modules:
        return
    try:
        import antenv
        mod = types.ModuleType("antenv.axon_hooks")
        _state = {"hook": None}
        mod.set_axon_ntff_profile_hook = lambda h: _state.__setitem__("hook", h)
        mod.get_axon_ntff_profile_hook = lambda: _state["hook"]
        sys.modules["antenv.axon_hooks"] = mod
        antenv.axon_hooks = mod
        from trn_agent_boot.trn_boot import _ntff_profile_via_ctypes
        mod.set_axon_ntff_profile_hook(
            _ntff_profile_via_ctypes("/opt/axon/libaxon_pjrt.so")
        )
    except Exception:
        pass


def _build_program():
    nc = bacc.Bacc(
        "TRN2",
        target_bir_lowering=False,
        debug=False,
        enable_asserts=False,
        num_devices=N_CORES,
    )
    xtd = nc.dram_tensor("xtd", [C, T], BF16, kind="ExternalInput").ap()
    xtd8 = nc.dram_tensor("xtd8", [128, CCH * T], FP8, kind="ExternalInput").ap()
    wqk = nc.dram_tensor("wqk", [128, CCH * FQK], FP8, kind="ExternalInput").ap()
    wv = nc.dram_tensor("wv", [128, CCH * HPC * E], BF16, kind="ExternalInput").ap()
    bqk = nc.dram_tensor("bqk", [128, CCH], F32, kind="ExternalInput").ap()
    bv = nc.dram_tensor("bv", [1, HPC * E], BF16, kind="ExternalInput").ap()
    wp = nc.dram_tensor("wp", [128, 3 * C], BF16, kind="ExternalInput").ap()
    onesd = nc.dram_tensor("onesd", [1, 128], BF16, kind="ExternalInput").ap()
    maskd = nc.dram_tensor("maskd", [128, 1024], BF16, kind="ExternalInput").ap()
    yp = nc.dram_tensor("yp", [T, C], BF16, kind="ExternalOutput").ap()

    with tile.TileContext(nc) as tc:
        _body(tc, nc, xtd, xtd8, wqk, wv, bqk, bv, wp, onesd, maskd, yp)

    nc.compile()
    return nc


def _body(tc, nc, xtd, xtd8, wqk, wv, bqk, bv, wp, onesd, maskd, yp):
    from contextlib import ExitStack

    ISCALE = 1.0 / float(np.sqrt(D))
    IDENT = mybir.ActivationFunctionType.Identity
    EXP = mybir.ActivationFunctionType.Exp

    with ExitStack() as es:
        persist = es.enter_context(tc.tile_pool(name="persist", bufs=1))
        # PSUM: scores 2x[128,1024] (4 banks) + fillers 2x[128,512] (2) +
        # AV accumulators 2x[65,512] (2) = 8 banks
        mmp = es.enter_context(tc.tile_pool(name="mmp", bufs=2, space="PSUM"))
        fmp = es.enter_context(tc.tile_pool(name="fmp", bufs=2, space="PSUM"))
        psyz = es.enter_context(tc.tile_pool(name="psyz", bufs=2, space="PSUM"))
        zpool = es.enter_context(tc.tile_pool(name="zpool", bufs=8))
        ypool = es.enter_context(tc.tile_pool(name="ypool", bufs=3))
        opool = es.enter_context(tc.tile_pool(name="opool", bufs=4))
        spool = es.enter_context(tc.tile_pool(name="spool", bufs=3))

        # ---- persistent SBUF tensors
        # fp8 Q/K path: weights packed [128, pair(3), tile(2), feat(768)],
        # x^T in both bf16 (V' path) and fp8 (DoubleRow QKV path)
        wqk_sb = persist.tile([128, CCH * FQK], FP8, tag="wqk", name="wqk_sb")
        xT8 = persist.tile([128, CCH * T], FP8, tag="xT8", name="xT8")
        wv_sb = persist.tile([128, CCH * HPC * E], BF16, tag="wv", name="wv_sb")
        wp_sb = persist.tile([128, 3 * C], BF16, tag="wp", name="wp_sb")
        bqk_sb = persist.tile([128, CCH], F32, tag="bqk", name="bqk_sb")
        bv_sb = persist.tile([1, HPC * E], BF16, tag="bv", name="bv_sb")
        ones_1x128 = persist.tile([1, 128], BF16, tag="ones128", name="ones_1x128")
        ones_f32 = persist.tile([1, 64], F32, tag="ones_f32", name="ones_f32")
        mask_sb = persist.tile([128, 1024], BF16, tag="mask", name="mask_sb")
        # x^T, Q^T/K^T: one wide tile each, cc-major stripes of length T
        xT = persist.tile([128, CCH * T], BF16, tag="xT", name="xT")
        qkt = persist.tile([128, CCH * T], BF16, tag="qkt", name="qkt")
        vp = [persist.tile([128, HPC * E], BF16, tag=f"vp{i}", name=f"vp{i}")
              for i in range(TT)]

        def xts(cc, lo, hi):
            return xT[:, cc * T + lo:cc * T + hi]

        def qs(ft, lo, hi):
            # ft 0..2: Q^T stripes, ft 3..5: K^T stripes
            return qkt[:, ft * T + lo:ft * T + hi]

        def x8s(p, q4):
            # contiguous [t=2 x n=512] block for pair p, chunk q4
            base = (p * QC + q4) * 1024
            return xT8[:, base:base + 1024]

        # ---- input DMAs: fp8 x chunk + fp8 weights first (QKV path), then
        # bf16 x chunk (V' path) and the rest
        for p in range(3):
            base = (p * QC) * 1024
            nc.sync.dma_start(x8s(p, 0), xtd8[:, base:base + 1024])
        nc.scalar.dma_start(wqk_sb[:], wqk[:])
        nc.scalar.dma_start(bqk_sb[:], bqk[:])
        for cc in range(CCH):
            nc.sync.dma_start(xts(cc, 0, 512), xtd[cc * 128:(cc + 1) * 128, 0:512])
        nc.scalar.dma_start(bv_sb[:], bv[:])
        nc.scalar.dma_start(ones_1x128[:], onesd[:])
        nc.vector.tensor_copy(ones_f32[:], ones_1x128[0:1, 0:64])
        for cc in range(CCH):
            nc.scalar.dma_start(
                wv_sb[:, cc * HPC * E:(cc + 1) * HPC * E],
                wv[:, cc * HPC * E:(cc + 1) * HPC * E])
        nc.scalar.dma_start(mask_sb[:], maskd[:])
        nc.scalar.dma_start(wp_sb[:], wp[:])

        def a_chunk(q4):
            for p in range(3):
                base = (p * QC + q4) * 1024
                nc.sync.dma_start(x8s(p, q4), xtd8[:, base:base + 1024])
            for cc in range(CCH):
                nc.sync.dma_start(
                    xts(cc, q4 * 512, (q4 + 1) * 512),
                    xtd[cc * 128:(cc + 1) * 128, q4 * 512:(q4 + 1) * 512],
                )

        # ---------------- filler closures (QKV / V' / proj) ----------------
        fill = deque()

        # fp8 DoubleRow: contraction 256 per matmul (two interleaved 128-deep
        # k-tiles).  wqk_sb is packed [128, pair, tile, feat]; xT8 stripes
        # give the rhs k-tile pair via a strided 3D view.
        DR = mybir.MatmulPerfMode.DoubleRow

        def wqk_slice(p, ft):
            base = (p * (FQK // 128) + ft) * 256
            return wqk_sb[:, base:base + 256].rearrange(
                "k (t m) -> k t m", t=2)

        def push_qkv(q4):
            # 6 groups: one feature tile each, 3 DoubleRow matmuls into a
            # single-bank [128,512] PSUM tile, then a DVE bias-add.
            for ft in range(FQK // 128):
                pt = [None]

                def mk_mm(p, ft=ft, pt=pt):
                    def go():
                        if pt[0] is None:
                            pt[0] = fmp.tile([128, 512], F32, tag="fm",
                                             name="qk_ps")
                        nc.tensor.matmul(
                            pt[0][:],
                            wqk_slice(p, ft),
                            x8s(p, q4).rearrange("k (t n) -> k t n", t=2),
                            start=(p == 0),
                            stop=(p == 2),
                            perf_mode=DR,
                        )
                    return go

                def mk_bias(ft=ft, pt=pt):
                    def go():
                        nc.vector.tensor_scalar_add(
                            qs(ft, q4 * 512, (q4 + 1) * 512),
                            pt[0][:],
                            bqk_sb[:, ft:ft + 1],
                        )
                    return go

                for p in range(3):
                    fill.append(mk_mm(p))
                fill.append(mk_bias())

        def push_vchunk(q4):
            # V' tiles for the 4 token blocks of chunk q4
            for j in range(4):
                tt = q4 * 4 + j
                pt = [None]

                def mk_mm(cc, tt=tt, pt=pt):
                    def go():
                        if pt[0] is None:
                            pt[0] = fmp.tile([128, 512], F32, tag="fm",
                                             name="v_ps")
                        nc.tensor.matmul(
                            pt[0][:, 0:HPC * E],
                            xts(cc, tt * 128, (tt + 1) * 128),
                            wv_sb[:, cc * HPC * E:(cc + 1) * HPC * E],
                            start=(cc == 0),
                            stop=False,
                        )
                    return go

                def mk_fin(tt=tt, pt=pt):
                    def go():
                        nc.tensor.matmul(
                            pt[0][:, 0:HPC * E], ones_1x128[:], bv_sb[:],
                            start=False, stop=True,
                        )
                        nc.vector.tensor_copy(vp[tt][:], pt[0][:, 0:HPC * E])
                    return go

                for cc in range(CCH):
                    fill.append(mk_mm(cc))
                fill.append(mk_fin())

        def push_proj(q4, yts):
            # output projection for chunk q4: 4 row-tiles x 2 column halves,
            # each half its own single-bank accumulation group + DVE copy
            for qt in range(4):
                ott = [None]
                for half in range(2):
                    pt = [None]

                    def mk_mm(hdc, half=half, qt=qt, pt=pt, ott=ott):
                        def go():
                            if pt[0] is None:
                                pt[0] = fmp.tile([128, 512], F32, tag="fm",
                                                 name="pj_ps")
                                if ott[0] is None:
                                    ott[0] = opool.tile([128, C], BF16,
                                                        tag="ot", name="ot")
                            nc.tensor.matmul(
                                pt[0][:, 0:384],
                                yts[hdc][:, qt * 128:(qt + 1) * 128],
                                wp_sb[:, (hdc * 2 + half) * 384:
                                      (hdc * 2 + half + 1) * 384],
                                start=(hdc == 0), stop=(hdc == 2),
                            )
                        return go

                    def mk_out(half=half, qt=qt, pt=pt, ott=ott):
                        def go():
                            nc.vector.tensor_copy(
                                ott[0][:, half * 384:(half + 1) * 384],
                                pt[0][:, 0:384])
                            if half == 1:
                                row = (q4 * 4 + qt) * 128
                                nc.sync.dma_start(
                                    yp[row:row + 128, :], ott[0][:])
                        return go

                    for hdc in range(3):
                        fill.append(mk_mm(hdc))
                    fill.append(mk_out())

        def pop_fill(n):
            for _ in range(n):
                if fill:
                    fill.popleft()()

        # ---------------- attention ----------------
        def attn_pair(q4, yts, hp, its_left):
            """Heads (2*hp, 2*hp+1): scores into one [128,1024] pair tile,
            one exp per kb (two on diagonal blocks), AV software-pipelined
            one iteration behind so the exp latency never stalls the PE."""
            h0, h1 = 2 * hp, 2 * hp + 1
            nkb = 4 * q4 + 4
            yz0 = psyz.tile([E, 512], F32, tag="yz", name="yz0")
            yz1 = psyz.tile([E, 512], F32, tag="yz", name="yz1")
            yzs = (yz0, yz1)

            order = list(range(nkb))

            def av(idx):
                kb = order[idx]
                off = max(0, kb * 128 - q4 * 512)
                w = 512 - off
                zt = zts[kb]
                for j, h in enumerate((h0, h1)):
                    nc.tensor.matmul(
                        yzs[j][:, off:512], vp[kb][:, h * E:(h + 1) * E],
                        zt[:, off + j * w:off + (j + 1) * w],
                        start=(idx == 0), stop=(idx == nkb - 1),
                    )

            zts = {}
            for idx, kb in enumerate(order):
                off = max(0, kb * 128 - q4 * 512)
                w = 512 - off
                diag = kb * 128 >= q4 * 512
                sp = mmp.tile([128, 1024], F32, tag="mm", name="sp")
                for j, h in enumerate((h0, h1)):
                    r0 = (h % 2) * 64
                    kt_stripe = (3 + h // 2) * T
                    qt_stripe = (h // 2) * T
                    nc.tensor.matmul(
                        sp[:, off + j * w:off + (j + 1) * w],
                        qkt[r0:r0 + 64,
                            kt_stripe + kb * 128:kt_stripe + (kb + 1) * 128],
                        qkt[r0:r0 + 64,
                            qt_stripe + q4 * 512 + off:
                            qt_stripe + (q4 + 1) * 512],
                        start=True, stop=True,
                    )
                zt = zpool.tile([128, 1024], BF16, tag="zt", name="zt")
                zts[kb] = zt
                # heads packed adjacently: one exp covers both live halves
                nc.scalar.activation(
                    zt[:, off:off + 2 * w], sp[:, off:off + 2 * w],
                    EXP, scale=ISCALE,
                )
                if diag:
                    for j in range(2):
                        nc.vector.tensor_mul(
                            zt[:, off + j * w:off + (j + 1) * w],
                            zt[:, off + j * w:off + (j + 1) * w],
                            mask_sb[:, 0:w])
                its_left -= 1
                if len(fill) > 16:
                    # keep ~16 closures in reserve: they drain right after
                    # the attention loop, keeping the PE busy through the
                    # final normalize chain of each chunk
                    pop_fill(-(-(len(fill) - 16) // max(1, its_left)))
                if idx > 1:
                    av(idx - 2)
            if nkb > 1:
                av(nkb - 2)
            av(nkb - 1)
            # cover the normalize chain below with queued PE work BEFORE
            # emitting it (later pops serialize behind it on the queues)
            pop_fill(16 if hp == 2 else 4)
            # normalize both heads with engine-interleaved chains.  den goes
            # to a base-0 tile: custom-DVE recip requires base partition 0.
            # For the very last pair (nothing left to hide the latency), the
            # copies run on the then-idle ACT engine and the broadcast is a
            # PE ones-matmul instead of the slow gpsimd broadcast.
            tail = (hp == 2) and (q4 == QC - 1)
            COPYF = mybir.ActivationFunctionType.Copy
            dens, rcs, ybs, bcs = [], [], [], []
            for j in range(2):
                den0 = spool.tile([1, 512], F32, tag="den0", name="den0")
                if tail:
                    nc.scalar.activation(den0[:], yzs[j][64:65, :], COPYF)
                else:
                    nc.vector.tensor_copy(den0[:], yzs[j][64:65, :])
                dens.append(den0)
            for j in range(2):
                rc = spool.tile([1, 512], F32, tag="rc", name="rc")
                nc.vector.reciprocal_approx_fast(rc[:], dens[j][:])
                rcs.append(rc)
                if tail:
                    bc_ps = fmp.tile([128, 512], F32, tag="fm", name="bc_ps")
                    nc.tensor.matmul(bc_ps[0:64, :], ones_f32[:],
                                     rc[:], start=True, stop=True)
                    bcs.append(bc_ps[0:64, :])
                else:
                    bc_sb = spool.tile([64, 512], F32, tag="bc_sb",
                                       name="bc_sb")
                    nc.gpsimd.partition_broadcast(bc_sb[:], rc[:])
                    bcs.append(bc_sb[:])
            for j in range(2):
                yb = spool.tile([64, 512], F32, tag="yb", name="yb")
                if tail:
                    nc.scalar.activation(yb[:], yzs[j][0:64, :], COPYF)
                else:
                    nc.vector.tensor_copy(yb[:], yzs[j][0:64, :])
                ybs.append(yb)
            for j, h in enumerate((h0, h1)):
                nc.vector.tensor_mul(
                    yts[h // 2][(h % 2) * 64:(h % 2) * 64 + 64, :],
                    ybs[j][:], bcs[j],
                )
            return its_left

        # ---------------- main schedule ----------------
        a_chunk(1)
        push_qkv(0)
        push_vchunk(0)
        pop_fill(len(fill))          # prologue: QKV+V for chunk 0 inline
        pending = None
        for q4 in range(QC):
            if pending is not None:
                push_proj(*pending)
            if q4 + 1 < QC:
                push_qkv(q4 + 1)
                push_vchunk(q4 + 1)
            if q4 + 2 < QC:
                a_chunk(q4 + 2)
            yts = [ypool.tile([128, 512], BF16, tag=f"yt{i}", name=f"yt{i}")
                   for i in range(3)]
            its_left = 3 * (4 * q4 + 4)
            for hp in range(3):
                its_left = attn_pair(q4, yts, hp, its_left)
            dbg = globals().get("_DBG")
            if dbg:
                for i in range(3):
                    nc.sync.dma_start(
                        dbg["yts_d"][(q4 * 3 + i) * 128:(q4 * 3 + i + 1) * 128, :],
                        yts[i][:])
            pending = (q4, yts)
        push_proj(*pending)
        pop_fill(len(fill))          # tail: final projection
        dbg = globals().get("_DBG")
        if dbg:
            nc.sync.dma_start(dbg["qkt_d"][:], qkt[:])
            nc.sync.dma_start(dbg["xt_d"][:], xT[:])
            for i in range(TT):
                nc.sync.dma_start(
                    dbg["vp_d"][i * 128:(i + 1) * 128, :], vp[i][:])


_PROGRAM = None


def _get_program():
    global _PROGRAM
    if _PROGRAM is None:
        _PROGRAM = _build_program()
    return _PROGRAM


def _pack_cc(w):
    # [C, F] -> [128, CCH*F] cc-major stripes
    F = w.shape[1]
    out = np.empty((128, CCH * F), dtype=NPBF)
    for cc in range(CCH):
        out[:, cc * F:(cc + 1) * F] = w[cc * 128:(cc + 1) * 128, :]
    return out


def _pack_wqk8(w):
    # [C=768, F=768] fp32 -> fp8 [128, (pair, ft, tile, m)] so each
    # DoubleRow lhsT [128, 2, 128] is a contiguous 256-col slice
    a = w.astype(NPF8).reshape(3, 2, 128, 6, 128)        # (p, t, k, ft, m)
    return np.ascontiguousarray(
        a.transpose(2, 0, 3, 1, 4).reshape(128, 4608))   # (k, p, ft, t, m)


def _pack_x8(xt):
    # x^T [768, 2048] -> fp8 [128, (pair, q4, tile, n)] so each DoubleRow
    # rhs [128, 2, 512] is a contiguous 1024-col slice
    a = xt.astype(NPF8).reshape(3, 2, 128, 4, 512)       # (p, t, k, q4, n)
    return np.ascontiguousarray(
        a.transpose(2, 0, 3, 1, 4).reshape(128, 12288))  # (k, p, q4, t, n)


def _pad_wv(wv):
    out = np.zeros((C, HPC * E), dtype=np.float32)
    for h in range(HPC):
        out[:, h * E:h * E + D] = wv[:, h * D:(h + 1) * D]
    return out


def _pad_bv(bv):
    out = np.zeros((HPC * E,), dtype=NPBF)
    for h in range(HPC):
        out[h * E:h * E + D] = bv[h * D:(h + 1) * D].astype(NPBF)
        out[h * E + D] = 1.0
    return out


def kernel(x, W_attn, b_attn, W_proj, b_proj):
    x = np.ascontiguousarray(x, dtype=np.float32)
    W_attn = np.ascontiguousarray(W_attn, dtype=np.float32)
    b_attn = np.ascontiguousarray(b_attn, dtype=np.float32)
    W_proj = np.ascontiguousarray(W_proj, dtype=np.float32)
    b_proj = np.ascontiguousarray(b_proj, dtype=np.float32)

    nc = _get_program()
    ones_const = np.ones((1, 128), dtype=NPBF)
    mask1 = np.triu(np.ones((128, 512), np.float32))
    mask_const = np.concatenate([mask1, mask1], axis=1).astype(NPBF)

    in_maps = []
    for core in range(N_CORES):
        b, g = core // 2, core % 2
        qcols = slice(384 * g, 384 * (g + 1))
        kcols = slice(768 + 384 * g, 768 + 384 * (g + 1))
        vcols = slice(1536 + 384 * g, 1536 + 384 * (g + 1))
        wqk_full = np.concatenate(
            [W_attn[:, qcols], W_attn[:, kcols]], axis=1)
        bqk_full = np.concatenate([b_attn[qcols], b_attn[kcols]])
        xt_host = np.ascontiguousarray(x[b].T)
        in_maps.append({
            "xtd": xt_host.astype(NPBF),
            "xtd8": _pack_x8(xt_host),
            "wqk": _pack_wqk8(wqk_full),
            "wv": _pack_cc(_pad_wv(W_attn[:, vcols]).astype(NPBF)),
            "bqk": np.ascontiguousarray(
                bqk_full.reshape(CCH, 128).T.astype(np.float32)),
            "bv": _pad_bv(b_attn[vcols])[None, :],
            "wp": np.ascontiguousarray(np.concatenate(
                [W_proj[384 * g + i * 128:384 * g + (i + 1) * 128, :]
                 .astype(NPBF) for i in range(3)], axis=1)),
            "onesd": ones_const,
            "maskd": mask_const,
        })

    trace = bool(int(os.environ.get("KBENCH_TRACE", "0")))
    if trace:
        _install_ntff_hook()
    res = run_bass_kernel_spmd(
        nc, in_maps, list(range(N_CORES)), trace=trace,
    )
    kernel.last_exec_time_ns = res.exec_time_ns

    out = np.empty((B, T, C), dtype=np.float32)
    for b in range(B):
        out[b] = (res.results[2 * b]["yp"].astype(np.float32)
                  + res.results[2 * b + 1]["yp"].astype(np.float32) + b_proj)
    return out
